# revision 24
# baseline (speedup 1.0000x reference)
# Trainium2 Bass kernel for nn_CapsuleLayer_62706522521966.
#
# Math: the reference's routing loop is dead code — softmax over a singleton
# axis (b_log is [I, O, 1], softmax on axis=2) yields all-ones coupling
# coefficients on every iteration, so the output is exactly
#     out = squash(einsum('bic,iocu->bou', x, w))[:, :, None, :]
# i.e. a single [B, I*C] @ [I*C, O*U] matmul followed by a tiny squash.
#
# Sharding: the O=32 output-capsule dim is split across the 8 NeuronCores
# (4 capsules each). Each core reads its own slice of w plus a replicated
# x^T — no collectives; the host concatenates the 8 slices.
#
# Perf notes:
#  - Matmul operands are cast to fp16 on the host (PSUM still accumulates
#    fp32): fp32 PE matmul is emulated as 2 HW matmuls (hi/lo) and fp32
#    doubles DMA bytes. fp16 keeps max rel err ~4e-4.
#  - Both operands are pre-permuted host-side into partition-major layouts
#    so every DMA reads contiguous HBM per partition.
#  - M=32 only fills a quarter of the PE array, so k-chunks are packed
#    4-at-a-time into the four 32-column groups (tile_position col-tiling),
#    accumulating into four partition slices of one PSUM bank; a final
#    [128->32] fold matmul with a stacked-identity lhsT sums the slices.
#  - w DMAs alternate between the SP and ACT HWDGE rings (issue is FIFO per
#    ring); x goes through SWDGE (gpsimd). First two w tiles are halved so
#    the PE starts after ~0.6 MB instead of ~1.5 MB.

from contextlib import ExitStack

import numpy as np

import concourse.bass as bass  # noqa: F401  (registers AP machinery)
import concourse.tile as tile
from concourse import bacc, mybir
from concourse.bass_utils import run_bass_kernel_spmd

B, I, O, C, U = 32, 2048, 32, 16, 32
N_CORES = 8
O_PER = O // N_CORES            # 4 output capsules per core
N = O_PER * U                   # 128 free (n) elements per core
K = I * C                       # 32768 contraction length
P = 128                         # SBUF partitions per k-chunk
KC = K // P                     # 256 k-chunks
XG = 64                         # k-chunks per x DMA tile (512 KB fp16)
# w DMA tiles as (first_chunk, n_chunks): half tiles at the ends — fast ramp
# at the start, shorter completion-latency exposure at the end
W_TILES = (
    [(0, 16), (16, 16)]
    + [(32 + 32 * k, 32) for k in range(6)]
    + [(224, 16), (240, 16)]
)
F32 = mybir.dt.float32
F16 = mybir.dt.float16
NP_IN = np.float16

_NC_CACHE: dict = {}


def _build_nc():
    nc = bacc.Bacc("TRN2", target_bir_lowering=False, debug=False)

    xt = nc.dram_tensor("xt", [P, KC * B], F16, kind="ExternalInput")
    wt = nc.dram_tensor("wt", [P, KC * N], F16, kind="ExternalInput")
    id4 = nc.dram_tensor("id4", [P, B], F16, kind="ExternalInput")
    out_d = nc.dram_tensor("out", [B, N], F32, kind="ExternalOutput")

    with tile.TileContext(nc) as tc:
        with ExitStack() as ctx:
            xpool = ctx.enter_context(tc.tile_pool(name="xpool", bufs=4))
            wpool = ctx.enter_context(tc.tile_pool(name="wpool", bufs=10))
            cpool = ctx.enter_context(tc.tile_pool(name="cpool", bufs=1))
            pspool = ctx.enter_context(
                tc.tile_pool(name="pspool", bufs=1, space="PSUM")
            )
            spool = ctx.enter_context(tc.tile_pool(name="spool", bufs=1))

            # four 32-partition accumulator slices in one PSUM bank
            pc = pspool.tile([P, N], F32)
            x_tiles = []
            first = True
            for c0, cnt in W_TILES:
                if c0 % XG == 0:
                    xi = c0 // XG
                    x_t = xpool.tile([P, XG, B], F16)
                    nc.scalar.dma_start(
                        out=x_t,
                        in_=xt[:, xi * XG * B : (xi + 1) * XG * B].rearrange(
                            "p (c b) -> p c b", b=B
                        ),
                    )
                    x_tiles.append(x_t)
                w_full = wpool.tile([P, 32 * N], F16, tag="w_t", name="w_t")
                w_t = w_full[:, : cnt * N]
                nc.sync.dma_start(
                    out=w_t, in_=wt[:, c0 * N : (c0 + cnt) * N]
                )
                if first:
                    # issued after the first x/w DMAs so they hit the rings
                    # first; still early enough to overlap the stream phase.
                    first = False
                    id_sb = cpool.tile([P, B], F16)
                    nc.scalar.dma_start(out=id_sb, in_=id4[:, :])
                    # Preload the Sqrt ACT table while PE/DMA do the real
                    # work, so the epilogue doesn't pay the ~1.3us load.
                    warm = spool.tile([1, 1], F32)
                    nc.vector.memset(warm, 1.0)
                    warm2 = spool.tile([1, 1], F32)
                    nc.scalar.sqrt(warm2, warm)
                for g in range(cnt):
                    c = c0 + g
                    j = c % 4
                    nc.tensor.matmul(
                        pc[32 * j : 32 * (j + 1), :],
                        lhsT=x_tiles[c // XG][:, c % XG, :],
                        rhs=w_t[:, g * N : (g + 1) * N],
                        start=(c < 4),
                        stop=(c >= KC - 4),
                        tile_position=(0, 32 * j),
                    )

            # fold the 4 partition slices: s = ID4^T @ pc_sb (fp16 weights are
            # exact 0/1; pc values get one fp16 rounding, ~5e-4 rel)
            pc_sb = spool.tile([P, N], F16)
            nc.vector.tensor_copy(pc_sb, pc)
            ps = pspool.tile([B, N], F32)
            nc.tensor.matmul(ps, lhsT=id_sb, rhs=pc_sb, start=True, stop=True)

            # squash: v = s * n / (1 + n^2), n = ||s|| over the unit dim.
            # ACT computes the per-o sum of squares straight off PSUM while
            # DVE copies s out; then a 3D broadcast multiply forms v.
            s_sb = spool.tile([B, N], F32)
            nc.vector.tensor_copy(s_sb, ps)
            sq = spool.tile([B, N], F32)
            ssq = spool.tile([B, O_PER], F32)
            for o in range(O_PER):
                nc.scalar.activation(
                    out=sq[:, o * U : (o + 1) * U],
                    in_=ps[:, o * U : (o + 1) * U],
                    func=mybir.ActivationFunctionType.Square,
                    accum_out=ssq[:, o : o + 1],
                )
            nrm = spool.tile([B, O_PER], F32)
            nc.scalar.sqrt(nrm, ssq)
            den = spool.tile([B, O_PER], F32)
            nc.vector.tensor_scalar_add(den, ssq, 1.0)
            rden = spool.tile([B, O_PER], F32)
            nc.vector.reciprocal(rden, den)
            fac = spool.tile([B, O_PER], F32)
            nc.vector.tensor_mul(fac, nrm, rden)
            v = spool.tile([B, O_PER, U], F32)
            fac_b = bass.AP(
                tensor=fac.tensor,
                offset=fac.offset,
                ap=[fac.ap[0], fac.ap[1], [0, U]],
            )
            nc.vector.tensor_mul(
                v, s_sb.rearrange("b (o u) -> b o u", u=U), fac_b
            )
            nc.sync.dma_start(
                out=out_d[:, :], in_=v.rearrange("b o u -> b (o u)")
            )

    nc.compile()
    return nc


def _build_nc_raw():
    """Hand-synchronized raw-bass variant: same dataflow as _build_nc but
    without the Tile framework's preamble/shutdown overhead (~7us + ~8us)."""
    nc = bass.Bass("TRN2", target_bir_lowering=False)

    RG = 16                     # k-chunks per w DMA (512 KB fp16)
    NT = KC // RG               # 16 w tiles
    RB = 8                      # w slot ring depth
    XH = KC // 2                # x loaded in two 1MB halves

    xt = nc.dram_tensor("xt", [P, KC * B], F16, kind="ExternalInput")
    wt = nc.dram_tensor("wt", [P, KC * N], F16, kind="ExternalInput")
    id4 = nc.dram_tensor("id4", [P, B], F16, kind="ExternalInput")
    out_d = nc.dram_tensor("out", [B, N], F32, kind="ExternalOutput")

    x_sb = nc.alloc_sbuf_tensor("x_sb", [P, KC * B], F16)
    w_sb = nc.alloc_sbuf_tensor("w_sb", [P, RB * RG * N], F16)
    id_sb = nc.alloc_sbuf_tensor("id_sb", [P, B], F16)
    pc_sb = nc.alloc_sbuf_tensor("pc_sb", [P, N], F16)
    warm = nc.alloc_sbuf_tensor("warm", [1, 3], F32)
    s_sb = nc.alloc_sbuf_tensor("s_sb", [B, N], F32)
    sqt = nc.alloc_sbuf_tensor("sqt", [B, N], F32)
    ssq = nc.alloc_sbuf_tensor("ssq", [B, O_PER], F32)
    nrm = nc.alloc_sbuf_tensor("nrm", [B, O_PER], F32)
    den = nc.alloc_sbuf_tensor("den", [B, O_PER], F32)
    rden = nc.alloc_sbuf_tensor("rden", [B, O_PER], F32)
    fac = nc.alloc_sbuf_tensor("fac", [B, O_PER], F32)
    v_sb = nc.alloc_sbuf_tensor("v_sb", [B, N], F32)

    pc = nc.alloc_psum_tensor("pc", [P, N], F32)
    ps = nc.alloc_psum_tensor("ps", [B, N], F32)

    # one sem per w tile / x half: HWDGE completions across the two HW
    # sub-queues are not FIFO, so a shared counting sem is racy
    s_ws = [nc.alloc_semaphore(f"s_w{t}") for t in range(NT)]
    s_xs = [nc.alloc_semaphore(f"s_x{h}") for h in range(2)]
    s_misc = nc.alloc_semaphore("s_misc")
    s_consts = nc.alloc_semaphore("s_consts")
    s_pe = nc.alloc_semaphore("s_pe")
    s_wu = nc.alloc_semaphore("s_wu")
    s_cp = nc.alloc_semaphore("s_cp")
    s_fold = nc.alloc_semaphore("s_fold")
    s_nrm = nc.alloc_semaphore("s_nrm")
    s_v = nc.alloc_semaphore("s_v")
    s_ve = nc.alloc_semaphore("s_ve")
    s_out = nc.alloc_semaphore("s_out")

    x_view = x_sb[:, :].rearrange("p (c b) -> p c b", b=B)
    s3d = s_sb[:, :].rearrange("b (o u) -> b o u", u=U)
    v3d = v_sb[:, :].rearrange("b (o u) -> b o u", u=U)
    fac_ap = fac[:, :]
    fac_b = bass.AP(
        tensor=fac_ap.tensor,
        offset=fac_ap.offset,
        ap=[fac_ap.ap[0], fac_ap.ap[1], [0, U]],
    )

    with nc.Block() as block:

        @block.sync
        def _(sync):
            for t in range(NT):
                if t >= RB:
                    sync.wait_ge(s_pe, t - RB + 1)
                sl = t % RB
                sync.dma_start(
                    out=w_sb[:, sl * RG * N : (sl + 1) * RG * N],
                    in_=wt[:, t * RG * N : (t + 1) * RG * N],
                ).then_inc(s_ws[t], 16)
            sync.wait_ge(s_v, 1)
            sync.dma_start(out=out_d[:, :], in_=v_sb[:, :]).then_inc(s_out, 16)
            sync.wait_ge(s_out, 16)

        @block.gpsimd
        def _(gpsimd):
            # stands in for the stripped start barrier: signals that the
            # framework const-AP memsets (emitted earlier on this engine)
            # have retired before ACT reads a const bias
            gpsimd.wait_ge(s_consts, 0).then_inc(s_consts, 1)

        @block.scalar
        def _(scalar):
            for h in range(2):
                scalar.dma_start(
                    out=x_sb[:, h * XH * B : (h + 1) * XH * B],
                    in_=xt[:, h * XH * B : (h + 1) * XH * B],
                ).then_inc(s_xs[h], 16)
            scalar.dma_start(out=id_sb[:, :], in_=id4[:, :]).then_inc(s_misc, 16)
            # preload the Sqrt ACT table during the stream phase (warming a
            # SECOND function here crashes on HW — see probe_raw bisect)
            scalar.wait_ge(s_wu, 1)
            scalar.wait_ge(s_consts, 1)
            nc.scalar.sqrt(warm[:, 2:3], warm[:, 0:1])
            # epilogue: n = sqrt(ssq) once DVE has reduced the squares
            scalar.wait_ge(s_ve, 3)
            nc.scalar.sqrt(nrm[:, :], ssq[:, :]).then_inc(s_nrm, 1)

        @block.tensor
        def _(tensor):
            for t in range(NT):
                tensor.wait_ge(s_ws[t], 16)
                if t % (NT // 2) == 0:
                    tensor.wait_ge(s_xs[t // (NT // 2)], 16)
                sl = t % RB
                for g in range(RG):
                    c = t * RG + g
                    j = c % 4
                    inst = nc.tensor.matmul(
                        pc[32 * j : 32 * (j + 1), :],
                        lhsT=x_view[:, c, :],
                        rhs=w_sb[:, (sl * RG + g) * N : (sl * RG + g + 1) * N],
                        start=(c < 4),
                        stop=(c >= KC - 4),
                        tile_position=(0, 32 * j),
                        skip_group_check=True,
                    )
                    if g == RG - 1:
                        inst.then_inc(s_pe, 1)
            tensor.wait_ge(s_cp, 1)
            tensor.wait_ge(s_misc, 16)
            nc.tensor.matmul(
                ps[:, :], lhsT=id_sb[:, :], rhs=pc_sb[:, :], start=True, stop=True
            ).then_inc(s_fold, 1)

        @block.vector
        def _(vector):
            nc.vector.memset(warm[:, 0:1], 1.0).then_inc(s_wu, 1)
            vector.wait_ge(s_pe, NT)
            nc.vector.tensor_copy(pc_sb[:, :], pc[:, :]).then_inc(s_cp, 1)
            vector.wait_ge(s_fold, 1)
            nc.vector.tensor_copy(s_sb[:, :], ps[:, :]).then_inc(s_ve, 1)
            vector.wait_ge(s_ve, 1)
            nc.vector.tensor_mul(sqt[:, :], s_sb[:, :], s_sb[:, :]).then_inc(
                s_ve, 1
            )
            vector.wait_ge(s_ve, 2)
            nc.vector.reduce_sum(
                ssq[:, :],
                sqt[:, :].rearrange("b (o u) -> b o u", u=U),
                axis=mybir.AxisListType.X,
            ).then_inc(s_ve, 1)
            vector.wait_ge(s_ve, 3)
            nc.vector.tensor_scalar_add(den[:, :], ssq[:, :], 1.0).then_inc(
                s_ve, 1
            )
            vector.wait_ge(s_ve, 4)
            nc.vector.reciprocal(rden[:, :], den[:, :]).then_inc(s_ve, 1)
            vector.wait_ge(s_nrm, 1)
            vector.wait_ge(s_ve, 5)
            nc.vector.tensor_mul(fac[:, :], nrm[:, :], rden[:, :]).then_inc(
                s_ve, 1
            )
            vector.wait_ge(s_ve, 6)
            nc.vector.tensor_mul(v3d, s3d, fac_b).then_inc(s_v, 1)

    _strip_first_barrier(nc)
    return nc


def _strip_first_barrier(nc):
    """Remove the first all-engine barrier cluster (engine-start stagger eats
    ~3us inside it; this kernel's own semaphore graph makes it redundant —
    the only cross-engine preamble dependency, the const-AP memsets on Pool,
    is consumed ~30us later by the epilogue sqrt)."""
    removed = 0
    for bb in nc.main_func.blocks:
        before = len(bb.instructions)
        keep = [i for i in bb.instructions if "barrier_" not in i.concise()]
        if len(keep) != before:
            del bb.instructions[:]
            for i in keep:
                bb.instructions.append(i)
            removed += before - len(keep)
    assert removed == 20, f"expected to remove 20 barrier insts, got {removed}"


def _get_nc():
    import os

    impl = os.environ.get("KERNEL_IMPL", "raw")
    key = f"nc_{impl}"
    if key not in _NC_CACHE:
        _NC_CACHE[key] = _build_nc_raw() if impl == "raw" else _build_nc()
    return _NC_CACHE[key]


def _prep_inputs(x: np.ndarray, w: np.ndarray):
    x = np.ascontiguousarray(x, dtype=np.float32)
    w = np.ascontiguousarray(w, dtype=np.float32)
    # x^T in partition-major layout: xt[p, ck, b] = x_flat[b, ck*128 + p]
    x_flat = x.reshape(B, K)
    xt_host = np.ascontiguousarray(
        x_flat.T.reshape(KC, P, B).transpose(1, 0, 2), dtype=NP_IN
    ).reshape(P, KC * B)
    id4_host = np.tile(np.eye(B, dtype=np.float16), (P // B, 1))
    in_maps = []
    for j in range(N_CORES):
        wsh = w[:, j * O_PER : (j + 1) * O_PER]  # [I, O_PER, C, U]
        # wt[p=(i_sub,c), ck, n=(o,u)] = w[ck*8+i_sub, o, c, u]
        wt_host = np.ascontiguousarray(
            wsh.reshape(KC, P // C, O_PER, C, U).transpose(1, 3, 0, 2, 4),
            dtype=NP_IN,
        ).reshape(P, KC * N)
        in_maps.append({"xt": xt_host, "wt": wt_host, "id4": id4_host})
    return in_maps


def run(inputs: dict, **spmd_kwargs):
    """Build+run the SPMD kernel; returns (full_output, BassKernelResults)."""
    nc = _get_nc()
    in_maps = _prep_inputs(inputs["x"], inputs["w"])
    res = run_bass_kernel_spmd(nc, in_maps, list(range(N_CORES)), **spmd_kwargs)
    parts = [res.results[j]["out"].reshape(B, O_PER, U) for j in range(N_CORES)]
    v = np.concatenate(parts, axis=1)  # [B, O, U]
    return np.ascontiguousarray(v[:, :, None, :]).astype(np.float32), res


def kernel(x: np.ndarray, w: np.ndarray) -> np.ndarray:
    out, _ = run({"x": x, "w": w})
    return out


# revision 25
# speedup vs baseline: 1.0718x; 1.0718x over previous
# Trainium2 Bass kernel for nn_CapsuleLayer_62706522521966.
#
# Math: the reference's routing loop is dead code — softmax over a singleton
# axis (b_log is [I, O, 1], softmax on axis=2) yields all-ones coupling
# coefficients on every iteration, so the output is exactly
#     out = squash(einsum('bic,iocu->bou', x, w))[:, :, None, :]
# i.e. a single [B, I*C] @ [I*C, O*U] matmul followed by a tiny squash.
#
# Sharding: the O=32 output-capsule dim is split across the 8 NeuronCores
# (4 capsules each). Each core reads its own slice of w plus a replicated
# x^T — no collectives; the host concatenates the 8 slices.
#
# Perf notes:
#  - Matmul operands are cast to fp16 on the host (PSUM still accumulates
#    fp32): fp32 PE matmul is emulated as 2 HW matmuls (hi/lo) and fp32
#    doubles DMA bytes. fp16 keeps max rel err ~4e-4.
#  - Both operands are pre-permuted host-side into partition-major layouts
#    so every DMA reads contiguous HBM per partition.
#  - M=32 only fills a quarter of the PE array, so k-chunks are packed
#    4-at-a-time into the four 32-column groups (tile_position col-tiling),
#    accumulating into four partition slices of one PSUM bank; a final
#    [128->32] fold matmul with a stacked-identity lhsT sums the slices.
#  - w DMAs alternate between the SP and ACT HWDGE rings (issue is FIFO per
#    ring); x goes through SWDGE (gpsimd). First two w tiles are halved so
#    the PE starts after ~0.6 MB instead of ~1.5 MB.

from contextlib import ExitStack

import numpy as np

import concourse.bass as bass  # noqa: F401  (registers AP machinery)
import concourse.tile as tile
from concourse import bacc, mybir
from concourse.bass_utils import run_bass_kernel_spmd

B, I, O, C, U = 32, 2048, 32, 16, 32
N_CORES = 8
O_PER = O // N_CORES            # 4 output capsules per core
N = O_PER * U                   # 128 free (n) elements per core
K = I * C                       # 32768 contraction length
P = 128                         # SBUF partitions per k-chunk
KC = K // P                     # 256 k-chunks
XG = 64                         # k-chunks per x DMA tile (512 KB fp16)
# w DMA tiles as (first_chunk, n_chunks): half tiles at the ends — fast ramp
# at the start, shorter completion-latency exposure at the end
W_TILES = (
    [(0, 16), (16, 16)]
    + [(32 + 32 * k, 32) for k in range(6)]
    + [(224, 16), (240, 16)]
)
F32 = mybir.dt.float32
F16 = mybir.dt.float16
NP_IN = np.float16

_NC_CACHE: dict = {}


def _build_nc():
    nc = bacc.Bacc("TRN2", target_bir_lowering=False, debug=False)

    xt = nc.dram_tensor("xt", [P, KC * B], F16, kind="ExternalInput")
    wt = nc.dram_tensor("wt", [P, KC * N], F16, kind="ExternalInput")
    id4 = nc.dram_tensor("id4", [P, B], F16, kind="ExternalInput")
    out_d = nc.dram_tensor("out", [B, N], F32, kind="ExternalOutput")

    with tile.TileContext(nc) as tc:
        with ExitStack() as ctx:
            xpool = ctx.enter_context(tc.tile_pool(name="xpool", bufs=4))
            wpool = ctx.enter_context(tc.tile_pool(name="wpool", bufs=10))
            cpool = ctx.enter_context(tc.tile_pool(name="cpool", bufs=1))
            pspool = ctx.enter_context(
                tc.tile_pool(name="pspool", bufs=1, space="PSUM")
            )
            spool = ctx.enter_context(tc.tile_pool(name="spool", bufs=1))

            # four 32-partition accumulator slices in one PSUM bank
            pc = pspool.tile([P, N], F32)
            x_tiles = []
            first = True
            for c0, cnt in W_TILES:
                if c0 % XG == 0:
                    xi = c0 // XG
                    x_t = xpool.tile([P, XG, B], F16)
                    nc.scalar.dma_start(
                        out=x_t,
                        in_=xt[:, xi * XG * B : (xi + 1) * XG * B].rearrange(
                            "p (c b) -> p c b", b=B
                        ),
                    )
                    x_tiles.append(x_t)
                w_full = wpool.tile([P, 32 * N], F16, tag="w_t", name="w_t")
                w_t = w_full[:, : cnt * N]
                nc.sync.dma_start(
                    out=w_t, in_=wt[:, c0 * N : (c0 + cnt) * N]
                )
                if first:
                    # issued after the first x/w DMAs so they hit the rings
                    # first; still early enough to overlap the stream phase.
                    first = False
                    id_sb = cpool.tile([P, B], F16)
                    nc.scalar.dma_start(out=id_sb, in_=id4[:, :])
                    # Preload the Sqrt ACT table while PE/DMA do the real
                    # work, so the epilogue doesn't pay the ~1.3us load.
                    warm = spool.tile([1, 1], F32)
                    nc.vector.memset(warm, 1.0)
                    warm2 = spool.tile([1, 1], F32)
                    nc.scalar.sqrt(warm2, warm)
                for g in range(cnt):
                    c = c0 + g
                    j = c % 4
                    nc.tensor.matmul(
                        pc[32 * j : 32 * (j + 1), :],
                        lhsT=x_tiles[c // XG][:, c % XG, :],
                        rhs=w_t[:, g * N : (g + 1) * N],
                        start=(c < 4),
                        stop=(c >= KC - 4),
                        tile_position=(0, 32 * j),
                    )

            # fold the 4 partition slices: s = ID4^T @ pc_sb (fp16 weights are
            # exact 0/1; pc values get one fp16 rounding, ~5e-4 rel)
            pc_sb = spool.tile([P, N], F16)
            nc.vector.tensor_copy(pc_sb, pc)
            ps = pspool.tile([B, N], F32)
            nc.tensor.matmul(ps, lhsT=id_sb, rhs=pc_sb, start=True, stop=True)

            # squash: v = s * n / (1 + n^2), n = ||s|| over the unit dim.
            # ACT computes the per-o sum of squares straight off PSUM while
            # DVE copies s out; then a 3D broadcast multiply forms v.
            s_sb = spool.tile([B, N], F32)
            nc.vector.tensor_copy(s_sb, ps)
            sq = spool.tile([B, N], F32)
            ssq = spool.tile([B, O_PER], F32)
            for o in range(O_PER):
                nc.scalar.activation(
                    out=sq[:, o * U : (o + 1) * U],
                    in_=ps[:, o * U : (o + 1) * U],
                    func=mybir.ActivationFunctionType.Square,
                    accum_out=ssq[:, o : o + 1],
                )
            nrm = spool.tile([B, O_PER], F32)
            nc.scalar.sqrt(nrm, ssq)
            den = spool.tile([B, O_PER], F32)
            nc.vector.tensor_scalar_add(den, ssq, 1.0)
            rden = spool.tile([B, O_PER], F32)
            nc.vector.reciprocal(rden, den)
            fac = spool.tile([B, O_PER], F32)
            nc.vector.tensor_mul(fac, nrm, rden)
            v = spool.tile([B, O_PER, U], F32)
            fac_b = bass.AP(
                tensor=fac.tensor,
                offset=fac.offset,
                ap=[fac.ap[0], fac.ap[1], [0, U]],
            )
            nc.vector.tensor_mul(
                v, s_sb.rearrange("b (o u) -> b o u", u=U), fac_b
            )
            nc.sync.dma_start(
                out=out_d[:, :], in_=v.rearrange("b o u -> b (o u)")
            )

    nc.compile()
    return nc


def _build_nc_raw():
    """Hand-synchronized raw-bass variant: same dataflow as _build_nc but
    without the Tile framework's preamble/shutdown overhead (~7us + ~8us)."""
    nc = bass.Bass("TRN2", target_bir_lowering=False)

    RG = 16                     # k-chunks per w DMA (512 KB fp16)
    NT = KC // RG               # 16 w tiles
    RB = 8                      # w slot ring depth
    XH = KC // 2                # x loaded in two 1MB halves

    xt = nc.dram_tensor("xt", [P, KC * B], F16, kind="ExternalInput")
    wt = nc.dram_tensor("wt", [P, KC * N], F16, kind="ExternalInput")
    id4 = nc.dram_tensor("id4", [P, B], F16, kind="ExternalInput")
    out_d = nc.dram_tensor("out", [B, N], F32, kind="ExternalOutput")

    x_sb = nc.alloc_sbuf_tensor("x_sb", [P, KC * B], F16)
    w_sb = nc.alloc_sbuf_tensor("w_sb", [P, RB * RG * N], F16)
    id_sb = nc.alloc_sbuf_tensor("id_sb", [P, B], F16)
    pc_sb = nc.alloc_sbuf_tensor("pc_sb", [P, N], F16)
    warm = nc.alloc_sbuf_tensor("warm", [1, 3], F32)
    s_sb = nc.alloc_sbuf_tensor("s_sb", [B, N], F32)
    sqt = nc.alloc_sbuf_tensor("sqt", [B, N], F32)
    ssq = nc.alloc_sbuf_tensor("ssq", [B, O_PER], F32)
    nrm = nc.alloc_sbuf_tensor("nrm", [B, O_PER], F32)
    den = nc.alloc_sbuf_tensor("den", [B, O_PER], F32)
    rden = nc.alloc_sbuf_tensor("rden", [B, O_PER], F32)
    fac = nc.alloc_sbuf_tensor("fac", [B, O_PER], F32)
    v_sb = nc.alloc_sbuf_tensor("v_sb", [B, N], F32)

    pc = nc.alloc_psum_tensor("pc", [P, N], F32)
    ps = nc.alloc_psum_tensor("ps", [B, N], F32)

    # one sem per w tile / x half: HWDGE completions across the two HW
    # sub-queues are not FIFO, so a shared counting sem is racy
    s_ws = [nc.alloc_semaphore(f"s_w{t}") for t in range(NT)]
    s_xs = [nc.alloc_semaphore(f"s_x{h}") for h in range(2)]
    s_misc = nc.alloc_semaphore("s_misc")
    s_consts = nc.alloc_semaphore("s_consts")
    s_pe = nc.alloc_semaphore("s_pe")
    s_wu = nc.alloc_semaphore("s_wu")
    s_cp = nc.alloc_semaphore("s_cp")
    s_fold = nc.alloc_semaphore("s_fold")
    s_nrm = nc.alloc_semaphore("s_nrm")
    s_v = nc.alloc_semaphore("s_v")
    s_ve = nc.alloc_semaphore("s_ve")
    s_out = nc.alloc_semaphore("s_out")

    x_view = x_sb[:, :].rearrange("p (c b) -> p c b", b=B)
    s3d = s_sb[:, :].rearrange("b (o u) -> b o u", u=U)
    v3d = v_sb[:, :].rearrange("b (o u) -> b o u", u=U)
    fac_ap = fac[:, :]
    fac_b = bass.AP(
        tensor=fac_ap.tensor,
        offset=fac_ap.offset,
        ap=[fac_ap.ap[0], fac_ap.ap[1], [0, U]],
    )

    with nc.Block() as block:

        @block.sync
        def _(sync):
            for t in range(NT):
                if t >= RB:
                    sync.wait_ge(s_pe, t - RB + 1)
                sl = t % RB
                sync.dma_start(
                    out=w_sb[:, sl * RG * N : (sl + 1) * RG * N],
                    in_=wt[:, t * RG * N : (t + 1) * RG * N],
                ).then_inc(s_ws[t], 16)
            sync.wait_ge(s_v, 1)
            sync.dma_start(out=out_d[:, :], in_=v_sb[:, :]).then_inc(s_out, 16)
            sync.wait_ge(s_out, 16)

        @block.gpsimd
        def _(gpsimd):
            # stands in for the stripped start barrier: signals that the
            # framework const-AP memsets (emitted earlier on this engine)
            # have retired before ACT reads a const bias
            gpsimd.wait_ge(s_consts, 0).then_inc(s_consts, 1)

        @block.scalar
        def _(scalar):
            for h in range(2):
                scalar.dma_start(
                    out=x_sb[:, h * XH * B : (h + 1) * XH * B],
                    in_=xt[:, h * XH * B : (h + 1) * XH * B],
                ).then_inc(s_xs[h], 16)
            scalar.dma_start(out=id_sb[:, :], in_=id4[:, :]).then_inc(s_misc, 16)
            # preload the Sqrt ACT table during the stream phase (warming a
            # SECOND function here crashes on HW — see probe_raw bisect)
            scalar.wait_ge(s_wu, 1)
            scalar.wait_ge(s_consts, 1)
            nc.scalar.sqrt(warm[:, 2:3], warm[:, 0:1])
            # epilogue: n = sqrt(ssq) once DVE has reduced the squares
            scalar.wait_ge(s_ve, 3)
            nc.scalar.sqrt(nrm[:, :], ssq[:, :]).then_inc(s_nrm, 1)

        @block.tensor
        def _(tensor):
            for t in range(NT):
                tensor.wait_ge(s_ws[t], 16)
                if t % (NT // 2) == 0:
                    tensor.wait_ge(s_xs[t // (NT // 2)], 16)
                sl = t % RB
                for g in range(RG):
                    c = t * RG + g
                    j = c % 4
                    inst = nc.tensor.matmul(
                        pc[32 * j : 32 * (j + 1), :],
                        lhsT=x_view[:, c, :],
                        rhs=w_sb[:, (sl * RG + g) * N : (sl * RG + g + 1) * N],
                        start=(c < 4),
                        stop=(c >= KC - 4),
                        tile_position=(0, 32 * j),
                        skip_group_check=True,
                    )
                    if g == RG - 1:
                        inst.then_inc(s_pe, 1)
            tensor.wait_ge(s_cp, 1)
            tensor.wait_ge(s_misc, 16)
            nc.tensor.matmul(
                ps[:, :], lhsT=id_sb[:, :], rhs=pc_sb[:, :], start=True, stop=True
            ).then_inc(s_fold, 1)

        @block.vector
        def _(vector):
            nc.vector.memset(warm[:, 0:1], 1.0).then_inc(s_wu, 1)
            vector.wait_ge(s_pe, NT)
            nc.vector.tensor_copy(pc_sb[:, :], pc[:, :]).then_inc(s_cp, 1)
            vector.wait_ge(s_fold, 1)
            nc.vector.tensor_copy(s_sb[:, :], ps[:, :]).then_inc(s_ve, 1)
            vector.wait_ge(s_ve, 1)
            nc.vector.tensor_mul(sqt[:, :], s_sb[:, :], s_sb[:, :]).then_inc(
                s_ve, 1
            )
            vector.wait_ge(s_ve, 2)
            nc.vector.reduce_sum(
                ssq[:, :],
                sqt[:, :].rearrange("b (o u) -> b o u", u=U),
                axis=mybir.AxisListType.X,
            ).then_inc(s_ve, 1)
            vector.wait_ge(s_ve, 3)
            nc.vector.tensor_scalar_add(den[:, :], ssq[:, :], 1.0).then_inc(
                s_ve, 1
            )
            vector.wait_ge(s_ve, 4)
            nc.vector.reciprocal(rden[:, :], den[:, :]).then_inc(s_ve, 1)
            vector.wait_ge(s_nrm, 1)
            vector.wait_ge(s_ve, 5)
            nc.vector.tensor_mul(fac[:, :], nrm[:, :], rden[:, :]).then_inc(
                s_ve, 1
            )
            vector.wait_ge(s_ve, 6)
            nc.vector.tensor_mul(v3d, s3d, fac_b).then_inc(s_v, 1)

    _strip_first_barrier(nc)
    return nc


def _strip_first_barrier(nc):
    """Remove the first all-engine barrier cluster (engine-start stagger eats
    ~3us inside it; this kernel's own semaphore graph makes it redundant —
    the only cross-engine preamble dependency, the const-AP memsets on Pool,
    is consumed ~30us later by the epilogue sqrt)."""
    kill = []
    seen_drain = set()
    seen_ev = set()
    pl_ev = 0
    for bb in nc.main_func.blocks:
        for ins in bb.instructions:
            c = ins.concise()
            if "barrier_" not in c:
                continue
            eng = str(ins.engine)
            ty = type(ins).__name__
            if "Pool" in eng and ty == "InstEventSemaphore":
                if pl_ev < 2:
                    kill.append(ins)
                    pl_ev += 1
            elif ty == "InstDrain" and eng not in seen_drain:
                kill.append(ins)
                seen_drain.add(eng)
            elif ty == "InstEventSemaphore" and eng not in seen_ev:
                kill.append(ins)
                seen_ev.add(eng)
    kill_ids = {id(k) for k in kill}
    removed = 0
    for bb in nc.main_func.blocks:
        before = len(bb.instructions)
        keep = [i for i in bb.instructions if id(i) not in kill_ids]
        if len(keep) != before:
            del bb.instructions[:]
            for i in keep:
                bb.instructions.append(i)
            removed += before - len(keep)
    assert removed == 10, f"expected to remove 10 barrier insts, got {removed}"


def _get_nc():
    import os

    impl = os.environ.get("KERNEL_IMPL", "raw")
    key = f"nc_{impl}"
    if key not in _NC_CACHE:
        _NC_CACHE[key] = _build_nc_raw() if impl == "raw" else _build_nc()
    return _NC_CACHE[key]


def _prep_inputs(x: np.ndarray, w: np.ndarray):
    x = np.ascontiguousarray(x, dtype=np.float32)
    w = np.ascontiguousarray(w, dtype=np.float32)
    # x^T in partition-major layout: xt[p, ck, b] = x_flat[b, ck*128 + p]
    x_flat = x.reshape(B, K)
    xt_host = np.ascontiguousarray(
        x_flat.T.reshape(KC, P, B).transpose(1, 0, 2), dtype=NP_IN
    ).reshape(P, KC * B)
    id4_host = np.tile(np.eye(B, dtype=np.float16), (P // B, 1))
    in_maps = []
    for j in range(N_CORES):
        wsh = w[:, j * O_PER : (j + 1) * O_PER]  # [I, O_PER, C, U]
        # wt[p=(i_sub,c), ck, n=(o,u)] = w[ck*8+i_sub, o, c, u]
        wt_host = np.ascontiguousarray(
            wsh.reshape(KC, P // C, O_PER, C, U).transpose(1, 3, 0, 2, 4),
            dtype=NP_IN,
        ).reshape(P, KC * N)
        in_maps.append({"xt": xt_host, "wt": wt_host, "id4": id4_host})
    return in_maps


def run(inputs: dict, **spmd_kwargs):
    """Build+run the SPMD kernel; returns (full_output, BassKernelResults)."""
    nc = _get_nc()
    in_maps = _prep_inputs(inputs["x"], inputs["w"])
    res = run_bass_kernel_spmd(nc, in_maps, list(range(N_CORES)), **spmd_kwargs)
    parts = [res.results[j]["out"].reshape(B, O_PER, U) for j in range(N_CORES)]
    v = np.concatenate(parts, axis=1)  # [B, O, U]
    return np.ascontiguousarray(v[:, :, None, :]).astype(np.float32), res


def kernel(x: np.ndarray, w: np.ndarray) -> np.ndarray:
    out, _ = run({"x": x, "w": w})
    return out


# revision 29
# speedup vs baseline: 1.0825x; 1.0099x over previous
# Trainium2 Bass kernel for nn_CapsuleLayer_62706522521966.
#
# Math: the reference's routing loop is dead code — softmax over a singleton
# axis (b_log is [I, O, 1], softmax on axis=2) yields all-ones coupling
# coefficients on every iteration, so the output is exactly
#     out = squash(einsum('bic,iocu->bou', x, w))[:, :, None, :]
# i.e. a single [B, I*C] @ [I*C, O*U] matmul followed by a tiny squash.
#
# Sharding: the O=32 output-capsule dim is split across the 8 NeuronCores
# (4 capsules each). Each core reads its own slice of w plus a replicated
# x^T — no collectives; the host concatenates the 8 slices.
#
# Perf notes:
#  - Matmul operands are cast to fp16 on the host (PSUM still accumulates
#    fp32): fp32 PE matmul is emulated as 2 HW matmuls (hi/lo) and fp32
#    doubles DMA bytes. fp16 keeps max rel err ~4e-4.
#  - Both operands are pre-permuted host-side into partition-major layouts
#    so every DMA reads contiguous HBM per partition.
#  - M=32 only fills a quarter of the PE array, so k-chunks are packed
#    4-at-a-time into the four 32-column groups (tile_position col-tiling),
#    accumulating into four partition slices of one PSUM bank; a final
#    [128->32] fold matmul with a stacked-identity lhsT sums the slices.
#  - w DMAs alternate between the SP and ACT HWDGE rings (issue is FIFO per
#    ring); x goes through SWDGE (gpsimd). First two w tiles are halved so
#    the PE starts after ~0.6 MB instead of ~1.5 MB.

from contextlib import ExitStack

import numpy as np

import concourse.bass as bass  # noqa: F401  (registers AP machinery)
import concourse.tile as tile
from concourse import bacc, mybir
from concourse.bass_utils import run_bass_kernel_spmd

B, I, O, C, U = 32, 2048, 32, 16, 32
N_CORES = 8
O_PER = O // N_CORES            # 4 output capsules per core
N = O_PER * U                   # 128 free (n) elements per core
K = I * C                       # 32768 contraction length
P = 128                         # SBUF partitions per k-chunk
KC = K // P                     # 256 k-chunks
XG = 64                         # k-chunks per x DMA tile (512 KB fp16)
# w DMA tiles as (first_chunk, n_chunks): half tiles at the ends — fast ramp
# at the start, shorter completion-latency exposure at the end
W_TILES = (
    [(0, 16), (16, 16)]
    + [(32 + 32 * k, 32) for k in range(6)]
    + [(224, 16), (240, 16)]
)
F32 = mybir.dt.float32
F16 = mybir.dt.float16
NP_IN = np.float16

_NC_CACHE: dict = {}


def _build_nc():
    nc = bacc.Bacc("TRN2", target_bir_lowering=False, debug=False)

    xt = nc.dram_tensor("xt", [P, KC * B], F16, kind="ExternalInput")
    wt = nc.dram_tensor("wt", [P, KC * N], F16, kind="ExternalInput")
    id4 = nc.dram_tensor("id4", [P, B], F16, kind="ExternalInput")
    out_d = nc.dram_tensor("out", [B, N], F32, kind="ExternalOutput")

    with tile.TileContext(nc) as tc:
        with ExitStack() as ctx:
            xpool = ctx.enter_context(tc.tile_pool(name="xpool", bufs=4))
            wpool = ctx.enter_context(tc.tile_pool(name="wpool", bufs=10))
            cpool = ctx.enter_context(tc.tile_pool(name="cpool", bufs=1))
            pspool = ctx.enter_context(
                tc.tile_pool(name="pspool", bufs=1, space="PSUM")
            )
            spool = ctx.enter_context(tc.tile_pool(name="spool", bufs=1))

            # four 32-partition accumulator slices in one PSUM bank
            pc = pspool.tile([P, N], F32)
            x_tiles = []
            first = True
            for c0, cnt in W_TILES:
                if c0 % XG == 0:
                    xi = c0 // XG
                    x_t = xpool.tile([P, XG, B], F16)
                    nc.scalar.dma_start(
                        out=x_t,
                        in_=xt[:, xi * XG * B : (xi + 1) * XG * B].rearrange(
                            "p (c b) -> p c b", b=B
                        ),
                    )
                    x_tiles.append(x_t)
                w_full = wpool.tile([P, 32 * N], F16, tag="w_t", name="w_t")
                w_t = w_full[:, : cnt * N]
                nc.sync.dma_start(
                    out=w_t, in_=wt[:, c0 * N : (c0 + cnt) * N]
                )
                if first:
                    # issued after the first x/w DMAs so they hit the rings
                    # first; still early enough to overlap the stream phase.
                    first = False
                    id_sb = cpool.tile([P, B], F16)
                    nc.scalar.dma_start(out=id_sb, in_=id4[:, :])
                    # Preload the Sqrt ACT table while PE/DMA do the real
                    # work, so the epilogue doesn't pay the ~1.3us load.
                    warm = spool.tile([1, 1], F32)
                    nc.vector.memset(warm, 1.0)
                    warm2 = spool.tile([1, 1], F32)
                    nc.scalar.sqrt(warm2, warm)
                for g in range(cnt):
                    c = c0 + g
                    j = c % 4
                    nc.tensor.matmul(
                        pc[32 * j : 32 * (j + 1), :],
                        lhsT=x_tiles[c // XG][:, c % XG, :],
                        rhs=w_t[:, g * N : (g + 1) * N],
                        start=(c < 4),
                        stop=(c >= KC - 4),
                        tile_position=(0, 32 * j),
                    )

            # fold the 4 partition slices: s = ID4^T @ pc_sb (fp16 weights are
            # exact 0/1; pc values get one fp16 rounding, ~5e-4 rel)
            pc_sb = spool.tile([P, N], F16)
            nc.vector.tensor_copy(pc_sb, pc)
            ps = pspool.tile([B, N], F32)
            nc.tensor.matmul(ps, lhsT=id_sb, rhs=pc_sb, start=True, stop=True)

            # squash: v = s * n / (1 + n^2), n = ||s|| over the unit dim.
            # ACT computes the per-o sum of squares straight off PSUM while
            # DVE copies s out; then a 3D broadcast multiply forms v.
            s_sb = spool.tile([B, N], F32)
            nc.vector.tensor_copy(s_sb, ps)
            sq = spool.tile([B, N], F32)
            ssq = spool.tile([B, O_PER], F32)
            for o in range(O_PER):
                nc.scalar.activation(
                    out=sq[:, o * U : (o + 1) * U],
                    in_=ps[:, o * U : (o + 1) * U],
                    func=mybir.ActivationFunctionType.Square,
                    accum_out=ssq[:, o : o + 1],
                )
            nrm = spool.tile([B, O_PER], F32)
            nc.scalar.sqrt(nrm, ssq)
            den = spool.tile([B, O_PER], F32)
            nc.vector.tensor_scalar_add(den, ssq, 1.0)
            rden = spool.tile([B, O_PER], F32)
            nc.vector.reciprocal(rden, den)
            fac = spool.tile([B, O_PER], F32)
            nc.vector.tensor_mul(fac, nrm, rden)
            v = spool.tile([B, O_PER, U], F32)
            fac_b = bass.AP(
                tensor=fac.tensor,
                offset=fac.offset,
                ap=[fac.ap[0], fac.ap[1], [0, U]],
            )
            nc.vector.tensor_mul(
                v, s_sb.rearrange("b (o u) -> b o u", u=U), fac_b
            )
            nc.sync.dma_start(
                out=out_d[:, :], in_=v.rearrange("b o u -> b (o u)")
            )

    nc.compile()
    return nc


def _build_nc_raw():
    """Hand-synchronized raw-bass variant: same dataflow as _build_nc but
    without the Tile framework's preamble/shutdown overhead (~7us + ~8us)."""
    nc = bass.Bass("TRN2", target_bir_lowering=False)

    RG = 16                     # k-chunks per w DMA (512 KB fp16)
    # w resident in SBUF (8.4 MB): no slot ring, one buffer region per tile.
    # Last tile split in two so the final completion-receipt window covers
    # only 256 KB of matmuls.
    W_PLAN = [(i * RG, RG) for i in range(KC // RG - 1)] + [
        (KC - RG, RG // 2),
        (KC - RG // 2, RG // 2),
    ]
    NT = len(W_PLAN)
    XH = KC // 2                # x loaded in two 1MB halves

    xt = nc.dram_tensor("xt", [P, KC * B], F16, kind="ExternalInput")
    wt = nc.dram_tensor("wt", [P, KC * N], F16, kind="ExternalInput")
    id4 = nc.dram_tensor("id4", [P, B], F16, kind="ExternalInput")
    out_d = nc.dram_tensor("out", [B, N], F32, kind="ExternalOutput")

    x_sb = nc.alloc_sbuf_tensor("x_sb", [P, KC * B], F16)
    w_sb = nc.alloc_sbuf_tensor("w_sb", [P, KC * N], F16)
    id_sb = nc.alloc_sbuf_tensor("id_sb", [P, B], F16)
    pc_sb = nc.alloc_sbuf_tensor("pc_sb", [P, N], F16)
    warm = nc.alloc_sbuf_tensor("warm", [1, 3], F32)
    s_sb = nc.alloc_sbuf_tensor("s_sb", [B, N], F32)
    sqt = nc.alloc_sbuf_tensor("sqt", [B, N], F32)
    ssq = nc.alloc_sbuf_tensor("ssq", [B, O_PER], F32)
    nrm = nc.alloc_sbuf_tensor("nrm", [B, O_PER], F32)
    den = nc.alloc_sbuf_tensor("den", [B, O_PER], F32)
    rden = nc.alloc_sbuf_tensor("rden", [B, O_PER], F32)
    fac = nc.alloc_sbuf_tensor("fac", [B, O_PER], F32)
    v_sb = nc.alloc_sbuf_tensor("v_sb", [B, N], F32)

    pc = nc.alloc_psum_tensor("pc", [P, N], F32)
    ps = nc.alloc_psum_tensor("ps", [B, N], F32)

    # one sem per w tile / x half: HWDGE completions across the two HW
    # sub-queues are not FIFO, so a shared counting sem is racy
    s_ws = [nc.alloc_semaphore(f"s_w{t}") for t in range(NT)]
    s_xs = [nc.alloc_semaphore(f"s_x{h}") for h in range(2)]
    s_misc = nc.alloc_semaphore("s_misc")
    s_consts = nc.alloc_semaphore("s_consts")
    s_pe = nc.alloc_semaphore("s_pe")
    s_wu = nc.alloc_semaphore("s_wu")
    s_cp = nc.alloc_semaphore("s_cp")
    s_fold = nc.alloc_semaphore("s_fold")
    s_nrm = nc.alloc_semaphore("s_nrm")
    s_v = nc.alloc_semaphore("s_v")
    s_ve = nc.alloc_semaphore("s_ve")
    s_out = nc.alloc_semaphore("s_out")

    x_view = x_sb[:, :].rearrange("p (c b) -> p c b", b=B)
    s3d = s_sb[:, :].rearrange("b (o u) -> b o u", u=U)
    v3d = v_sb[:, :].rearrange("b (o u) -> b o u", u=U)
    fac_ap = fac[:, :]
    fac_b = bass.AP(
        tensor=fac_ap.tensor,
        offset=fac_ap.offset,
        ap=[fac_ap.ap[0], fac_ap.ap[1], [0, U]],
    )

    with nc.Block() as block:

        @block.sync
        def _(sync):
            for t, (c0, cnt) in enumerate(W_PLAN):
                sync.dma_start(
                    out=w_sb[:, c0 * N : (c0 + cnt) * N],
                    in_=wt[:, c0 * N : (c0 + cnt) * N],
                ).then_inc(s_ws[t], 16)
            sync.wait_ge(s_v, 1)
            sync.dma_start(out=out_d[:, :], in_=v_sb[:, :]).then_inc(s_out, 16)
            sync.wait_ge(s_out, 16)

        @block.gpsimd
        def _(gpsimd):
            # stands in for the stripped start barrier: signals that the
            # framework const-AP memsets (emitted earlier on this engine)
            # have retired before ACT reads a const bias
            gpsimd.wait_ge(s_consts, 0).then_inc(s_consts, 1)

        @block.scalar
        def _(scalar):
            for h in range(2):
                scalar.dma_start(
                    out=x_sb[:, h * XH * B : (h + 1) * XH * B],
                    in_=xt[:, h * XH * B : (h + 1) * XH * B],
                ).then_inc(s_xs[h], 16)
            scalar.dma_start(out=id_sb[:, :], in_=id4[:, :]).then_inc(s_misc, 16)
            # preload the Sqrt ACT table during the stream phase (warming a
            # SECOND function here crashes on HW — see probe_raw bisect)
            scalar.wait_ge(s_wu, 1)
            scalar.wait_ge(s_consts, 1)
            nc.scalar.sqrt(warm[:, 2:3], warm[:, 0:1])
            # epilogue: n = sqrt(ssq) once DVE has reduced the squares
            scalar.wait_ge(s_ve, 3)
            nc.scalar.sqrt(nrm[:, :], ssq[:, :]).then_inc(s_nrm, 1)

        @block.tensor
        def _(tensor):
            for t, (c0, cnt) in enumerate(W_PLAN):
                tensor.wait_ge(s_ws[t], 16)
                if c0 % XH == 0:
                    tensor.wait_ge(s_xs[c0 // XH], 16)
                for g in range(cnt):
                    c = c0 + g
                    j = c % 4
                    inst = nc.tensor.matmul(
                        pc[32 * j : 32 * (j + 1), :],
                        lhsT=x_view[:, c, :],
                        rhs=w_sb[:, c * N : (c + 1) * N],
                        start=(c < 4),
                        stop=(c >= KC - 4),
                        tile_position=(0, 32 * j),
                        skip_group_check=True,
                    )
                    if g == cnt - 1:
                        inst.then_inc(s_pe, 1)
            tensor.wait_ge(s_cp, 1)
            tensor.wait_ge(s_misc, 16)
            nc.tensor.matmul(
                ps[:, :], lhsT=id_sb[:, :], rhs=pc_sb[:, :], start=True, stop=True
            ).then_inc(s_fold, 1)

        @block.vector
        def _(vector):
            nc.vector.memset(warm[:, 0:1], 1.0).then_inc(s_wu, 1)
            vector.wait_ge(s_pe, NT)
            nc.vector.tensor_copy(pc_sb[:, :], pc[:, :]).then_inc(s_cp, 1)
            vector.wait_ge(s_fold, 1)
            nc.vector.tensor_copy(s_sb[:, :], ps[:, :]).then_inc(s_ve, 1)
            vector.wait_ge(s_ve, 1)
            nc.vector.tensor_mul(sqt[:, :], s_sb[:, :], s_sb[:, :]).then_inc(
                s_ve, 1
            )
            vector.wait_ge(s_ve, 2)
            nc.vector.reduce_sum(
                ssq[:, :],
                sqt[:, :].rearrange("b (o u) -> b o u", u=U),
                axis=mybir.AxisListType.X,
            ).then_inc(s_ve, 1)
            vector.wait_ge(s_ve, 3)
            nc.vector.tensor_scalar_add(den[:, :], ssq[:, :], 1.0).then_inc(
                s_ve, 1
            )
            vector.wait_ge(s_ve, 4)
            nc.vector.reciprocal(rden[:, :], den[:, :]).then_inc(s_ve, 1)
            vector.wait_ge(s_nrm, 1)
            vector.wait_ge(s_ve, 5)
            nc.vector.tensor_mul(fac[:, :], nrm[:, :], rden[:, :]).then_inc(
                s_ve, 1
            )
            vector.wait_ge(s_ve, 6)
            nc.vector.tensor_mul(v3d, s3d, fac_b).then_inc(s_v, 1)

    _strip_first_barrier(nc)
    return nc


def _strip_first_barrier(nc):
    """Remove the first all-engine barrier cluster (engine-start stagger eats
    ~3us inside it; this kernel's own semaphore graph makes it redundant —
    the only cross-engine preamble dependency, the const-AP memsets on Pool,
    is consumed ~30us later by the epilogue sqrt)."""
    kill = []
    seen_drain = set()
    seen_ev = set()
    pl_ev = 0
    for bb in nc.main_func.blocks:
        for ins in bb.instructions:
            c = ins.concise()
            if "barrier_" not in c:
                continue
            eng = str(ins.engine)
            ty = type(ins).__name__
            if "Pool" in eng and ty == "InstEventSemaphore":
                if pl_ev < 2:
                    kill.append(ins)
                    pl_ev += 1
            elif ty == "InstDrain" and eng not in seen_drain:
                kill.append(ins)
                seen_drain.add(eng)
            elif ty == "InstEventSemaphore" and eng not in seen_ev:
                kill.append(ins)
                seen_ev.add(eng)
    kill_ids = {id(k) for k in kill}
    removed = 0
    for bb in nc.main_func.blocks:
        before = len(bb.instructions)
        keep = [i for i in bb.instructions if id(i) not in kill_ids]
        if len(keep) != before:
            del bb.instructions[:]
            for i in keep:
                bb.instructions.append(i)
            removed += before - len(keep)
    assert removed == 10, f"expected to remove 10 barrier insts, got {removed}"


def _get_nc():
    import os

    impl = os.environ.get("KERNEL_IMPL", "raw")
    key = f"nc_{impl}"
    if key not in _NC_CACHE:
        _NC_CACHE[key] = _build_nc_raw() if impl == "raw" else _build_nc()
    return _NC_CACHE[key]


def _prep_inputs(x: np.ndarray, w: np.ndarray):
    x = np.ascontiguousarray(x, dtype=np.float32)
    w = np.ascontiguousarray(w, dtype=np.float32)
    # x^T in partition-major layout: xt[p, ck, b] = x_flat[b, ck*128 + p]
    x_flat = x.reshape(B, K)
    xt_host = np.ascontiguousarray(
        x_flat.T.reshape(KC, P, B).transpose(1, 0, 2), dtype=NP_IN
    ).reshape(P, KC * B)
    id4_host = np.tile(np.eye(B, dtype=np.float16), (P // B, 1))
    in_maps = []
    for j in range(N_CORES):
        wsh = w[:, j * O_PER : (j + 1) * O_PER]  # [I, O_PER, C, U]
        # wt[p=(i_sub,c), ck, n=(o,u)] = w[ck*8+i_sub, o, c, u]
        wt_host = np.ascontiguousarray(
            wsh.reshape(KC, P // C, O_PER, C, U).transpose(1, 3, 0, 2, 4),
            dtype=NP_IN,
        ).reshape(P, KC * N)
        in_maps.append({"xt": xt_host, "wt": wt_host, "id4": id4_host})
    return in_maps


def run(inputs: dict, **spmd_kwargs):
    """Build+run the SPMD kernel; returns (full_output, BassKernelResults)."""
    nc = _get_nc()
    in_maps = _prep_inputs(inputs["x"], inputs["w"])
    res = run_bass_kernel_spmd(nc, in_maps, list(range(N_CORES)), **spmd_kwargs)
    parts = [res.results[j]["out"].reshape(B, O_PER, U) for j in range(N_CORES)]
    v = np.concatenate(parts, axis=1)  # [B, O, U]
    return np.ascontiguousarray(v[:, :, None, :]).astype(np.float32), res


def kernel(x: np.ndarray, w: np.ndarray) -> np.ndarray:
    out, _ = run({"x": x, "w": w})
    return out
